# revision 14
# baseline (speedup 1.0000x reference)
"""Block-diagonal cross-attention + MLP for trn2, 8-core data-parallel.

v4: size-sorted graph banding + fp16 score matmuls + superpair batching.

- 128 graphs sorted by size desc into 4 bands of 32; band k gets a shared
  cap[k] = roundup(max size, 8). Each core takes 4 graphs from each band
  (one superpair/SP of 4 graphs per band) -> identical shapes across cores
  (SPMD) with ~30% less padding than a global cap.
- Scores in fp16 (4x the mantissa of bf16 at the same PE speed); E, values,
  weights, eT in bf16; PSUM/normalization in fp32; output fp32.
- Per SP: 8 score MMs into a 2-bank psum [cap, 1024] (4 blocks/bank, never
  crossing a bank), ONE exp ACT (strided) -> et bf16, 8 V MMs into the
  psum slot (values + mask-col rowsum), ONE reciprocal, ONE broadcast-AP
  normalize mul -> er bf16, 4 PE transposes -> tp psum, ONE fused
  evict+residual add -> eT.
- MLP over eT in <=512 chunks; bias1+relu alternates ACT/DVE; residual via
  identity matmul; per-chunk output DMA.
- 7 input DMAs + 3 output DMAs total, split across sync/gpsimd queues.
"""

from contextlib import ExitStack

import numpy as np
import ml_dtypes

BF16 = ml_dtypes.bfloat16

N_NODES = 8192
D = 64
G = 128
N_CORES = 8
GPC = G // N_CORES          # graphs per core = 16
VW = D + 1                  # value width incl. mask column
NBAND = 4                   # bands == superpairs per core
BANDG = G // NBAND          # graphs per band = 32

_PROGRAM_CACHE = {}


def _build_program(caps):
    import concourse.bass as bass
    import concourse.tile as tile
    from concourse import bacc, mybir

    fp32 = mybir.dt.float32
    bf16 = mybir.dt.bfloat16
    fp16 = mybir.dt.float16
    caps = list(caps)
    cap0 = max(caps)
    off = [4 * sum(caps[:k]) for k in range(NBAND)]
    ct = 4 * sum(caps)                  # total node columns per core
    nc = bacc.Bacc("TRN2", target_bir_lowering=False, debug=False)

    xs = nc.declare_dram_parameter("xs", [D, ct], fp16, isOutput=False)
    xt = nc.declare_dram_parameter("xt", [D, ct], fp16, isOutput=False)
    xp = nc.declare_dram_parameter("xp", [2 * D, ct], bf16, isOutput=False)
    vs = nc.declare_dram_parameter("vs", [cap0, GPC * VW], bf16, isOutput=False)
    vt = nc.declare_dram_parameter("vt", [cap0, GPC * VW], bf16, isOutput=False)
    # w1 | w2 | ident packed [128, 384] bf16; b1 | b2 packed [128, 2] fp32
    cwb = nc.declare_dram_parameter("cwb", [2 * D, 6 * D], bf16, isOutput=False)
    cbf = nc.declare_dram_parameter("cbf", [2 * D, 2], fp32, isOutput=False)
    # chunk-major output: band k's chunk lives at rows [128*k, 128*(k+1))
    outp = nc.declare_dram_parameter("outp", [NBAND * 2 * D, 4 * cap0], fp32,
                                     isOutput=True)

    AF = mybir.ActivationFunctionType
    ALU = mybir.AluOpType

    with tile.TileContext(nc) as tc, ExitStack() as ctx:
        singles = ctx.enter_context(tc.tile_pool(name="singles", bufs=1))
        epool = ctx.enter_context(tc.tile_pool(name="epool", bufs=3))
        work = ctx.enter_context(tc.tile_pool(name="work", bufs=3))
        opool = ctx.enter_context(tc.tile_pool(name="opool", bufs=3))

        sb_xs = singles.tile([D, ct], fp16, tag="xs")
        sb_xt = singles.tile([D, ct], fp16, tag="xt")
        sb_xp = singles.tile([2 * D, ct], bf16, tag="xp")
        sb_vs = singles.tile([cap0, GPC * VW], bf16, tag="vs")
        sb_vt = singles.tile([cap0, GPC * VW], bf16, tag="vt")
        sb_cw = singles.tile([2 * D, 6 * D], bf16, tag="cwb")
        sb_cb = singles.tile([2 * D, 2], fp32, tag="cbf")
        sb_eT = singles.tile([2 * D, ct], bf16, tag="eT")
        sb_h = singles.tile([2 * D, ct], bf16, tag="h")
        sb_w1 = sb_cw[:, 0:2 * D]
        sb_w2 = sb_cw[:, 2 * D:4 * D]
        sb_id = sb_cw[:, 4 * D:6 * D]
        sb_b1 = sb_cb[:, 0:1]
        sb_b2 = sb_cb[:, 1:2]

        # two HWDGE queues (sync + scalar); band-0 slices first so the
        # pipeline's first stages start as early as possible
        o1 = off[1]
        nc.sync.dma_start(out=sb_xs[:, 0:o1], in_=xs[:, 0:o1])
        nc.scalar.dma_start(out=sb_xt[:, 0:o1], in_=xt[:, 0:o1])
        nc.sync.dma_start(out=sb_vs[:, 0:4 * VW], in_=vs[:, 0:4 * VW])
        nc.scalar.dma_start(out=sb_vt[:, 0:4 * VW], in_=vt[:, 0:4 * VW])
        nc.sync.dma_start(out=sb_xp[:, 0:o1], in_=xp[:, 0:o1])
        nc.scalar.dma_start(out=sb_cw, in_=cwb[:, :])
        nc.sync.dma_start(out=sb_xs[:, o1:ct], in_=xs[:, o1:ct])
        nc.scalar.dma_start(out=sb_xt[:, o1:ct], in_=xt[:, o1:ct])
        nc.sync.dma_start(out=sb_vs[:, 4 * VW:], in_=vs[:, 4 * VW:])
        nc.scalar.dma_start(out=sb_vt[:, 4 * VW:], in_=vt[:, 4 * VW:])
        nc.sync.dma_start(out=sb_xp[:, o1:ct], in_=xp[:, o1:ct])
        nc.scalar.dma_start(out=sb_cb, in_=cbf[:, :])

        def scol(cap, j, d):
            return 512 * (j // 2) + ((j % 2) * 2 + d) * cap

        def ocol(j, d):
            return 512 * (j // 2) + ((j % 2) * 2 + d) * VW

        scs, ets, ers = {}, {}, {}
        with tc.tile_pool(name="ps_a", bufs=2, space="PSUM") as ps_a, \
             tc.tile_pool(name="ps_t", bufs=2, space="PSUM") as ps_t, \
             tc.tile_pool(name="ps_m", bufs=2, space="PSUM") as ps_m:
            # skewed pipeline: iter k emits scores_k/exp_k, V_{k-1},
            # tp/add/mlp-L1_{k-2}, mlp-L2_{k-3} so the PE stream never
            # waits on the scalar/vector stages
            for k in range(NBAND + 3):
                if k < NBAND:
                    cap = caps[k]
                    ob = off[k]
                    sc = ps_a.tile([cap, 1024], fp32, tag="sco")
                    scs[k] = sc
                    for j in range(4):
                        xsj = sb_xs[:, ob + j * cap: ob + (j + 1) * cap]
                        xtj = sb_xt[:, ob + j * cap: ob + (j + 1) * cap]
                        nc.tensor.matmul(sc[:, scol(cap, j, 0):scol(cap, j, 0) + cap],
                                         xsj, xtj, start=True, stop=True)
                        nc.tensor.matmul(sc[:, scol(cap, j, 1):scol(cap, j, 1) + cap],
                                         xtj, xsj, start=True, stop=True)
                    et = epool.tile([cap, 8 * cap], bf16, tag="E")
                    ets[k] = et
                    sc_str = sc.rearrange("p (a c) -> p a c", a=2)[:, :, 0:4 * cap]
                    nc.scalar.activation(out=et, in_=sc_str, func=AF.Exp)

                if 1 <= k <= NBAND:
                    kk = k - 1
                    cap = caps[kk]
                    sc, et = scs[kk], ets[kk]
                    # V matmuls reuse the score psum slot (WAR on exp)
                    for j in range(4):
                        vcol = (4 * kk + j) * VW
                        nc.tensor.matmul(
                            sc[:, ocol(j, 0):ocol(j, 0) + VW],
                            et[:, (2 * j + 1) * cap:(2 * j + 2) * cap],
                            sb_vt[0:cap, vcol:vcol + VW], start=True, stop=True)
                        nc.tensor.matmul(
                            sc[:, ocol(j, 1):ocol(j, 1) + VW],
                            et[:, (2 * j) * cap:(2 * j + 1) * cap],
                            sb_vs[0:cap, vcol:vcol + VW], start=True, stop=True)
                    o4 = sc.rearrange("p (a c) -> p a c", a=2)[:, :, 0:4 * VW] \
                           .rearrange("p a (b w) -> p a b w", w=VW)
                    rc = work.tile([cap, 8], fp32, tag="rc")
                    nc.vector.reciprocal(out=rc, in_=o4[:, :, 0:4, D:D + 1])
                    er = work.tile([cap, 8 * D], bf16, tag="er")
                    ers[kk] = er
                    rcb = rc.rearrange("p (a b) -> p a b", a=2).to_broadcast(
                        [cap, 2, 4, D])
                    nc.vector.tensor_mul(er.rearrange("p (a b w) -> p a b w",
                                                      a=2, w=D),
                                         o4[:, :, 0:4, 0:D], rcb)

                if 2 <= k <= NBAND + 1:
                    kk = k - 2
                    cap = caps[kk]
                    ob = off[kk]
                    er = ers.pop(kk)
                    tp = ps_t.tile([2 * D, 4 * cap], bf16, tag="tp")
                    for j in range(4):
                        nc.tensor.transpose(tp[:, j * cap:(j + 1) * cap],
                                            er[:, j * 2 * D:(j + 1) * 2 * D],
                                            sb_id[0:cap, 0:cap])
                    nc.vector.tensor_add(sb_eT[:, ob:ob + 4 * cap], tp,
                                         sb_xp[:, ob:ob + 4 * cap])
                    # MLP layer 1 for this band's chunk
                    w = 4 * cap
                    hp = ps_m.tile([2 * D, 4 * cap0], fp32, tag="m")
                    nc.tensor.matmul(hp[:, 0:w], sb_w1, sb_eT[:, ob:ob + w],
                                     start=True, stop=True)
                    if kk % 2 == 0:
                        nc.scalar.activation(out=sb_h[:, ob:ob + w],
                                             in_=hp[:, 0:w],
                                             func=AF.Relu, bias=sb_b1, scale=1.0)
                    else:
                        nc.vector.tensor_scalar(
                            out=sb_h[:, ob:ob + w], in0=hp[:, 0:w],
                            scalar1=sb_b1,
                            scalar2=0.0, op0=ALU.add, op1=ALU.max)

                if k >= 3:
                    kk = k - 3
                    cap = caps[kk]
                    ob = off[kk]
                    w = 4 * cap
                    op2 = ps_m.tile([2 * D, 4 * cap0], fp32, tag="m")
                    nc.tensor.matmul(op2[:, 0:w], sb_w2, sb_h[:, ob:ob + w],
                                     start=True, stop=False)
                    nc.tensor.matmul(op2[:, 0:w], sb_id, sb_eT[:, ob:ob + w],
                                     start=False, stop=True)
                    ot = opool.tile([2 * D, 4 * cap0], fp32, tag="out")
                    if kk % 2 == 0:
                        nc.scalar.activation(out=ot[:, 0:w], in_=op2[:, 0:w],
                                             func=AF.Identity, bias=sb_b2,
                                             scale=1.0)
                    else:
                        nc.vector.tensor_scalar_add(ot[:, 0:w], op2[:, 0:w],
                                                    sb_b2)
                    nc.sync.dma_start(out=outp[kk * 2 * D:(kk + 1) * 2 * D, 0:w],
                                      in_=ot[:, 0:w])

    nc.compile()
    return nc


def _plan(cnt_s, cnt_t):
    size = np.maximum(cnt_s, cnt_t)
    order = np.argsort(-size, kind="stable")
    bands = order.reshape(NBAND, BANDG)
    caps = tuple(int(-(-int(size[b].max()) // 8) * 8) for b in bands)
    core_of = np.empty(G, np.int64)
    band_of = np.empty(G, np.int64)
    slot_of = np.empty(G, np.int64)
    for k in range(NBAND):
        for c in range(N_CORES):
            for j in range(4):
                g = bands[k, c * 4 + j]
                core_of[g] = c
                band_of[g] = k
                slot_of[g] = j
    return caps, core_of, band_of, slot_of


def _shard_inputs(x_src, batch_src, x_tar, batch_tar, w1, b1, w2, b2, plan):
    caps, core_of, band_of, slot_of = plan
    bs = np.asarray(batch_src).astype(np.int64)
    bt = np.asarray(batch_tar).astype(np.int64)
    xsf = np.asarray(x_src, dtype=np.float32)
    xtf = np.asarray(x_tar, dtype=np.float32)
    cap0 = max(caps)
    offs = np.array([4 * sum(caps[:k]) for k in range(NBAND)], np.int64)
    capv = np.array(caps, np.int64)
    ct = int(4 * sum(caps))

    bnd_s = np.searchsorted(bs, np.arange(G + 1))
    bnd_t = np.searchsorted(bt, np.arange(G + 1))
    ws_ = np.arange(N_NODES) - bnd_s[bs]
    wt_ = np.arange(N_NODES) - bnd_t[bt]
    cs_, ct_ = core_of[bs], core_of[bt]
    ks_, kt_ = band_of[bs], band_of[bt]
    js_, jt_ = slot_of[bs], slot_of[bt]
    col_s = offs[ks_] + js_ * capv[ks_] + ws_
    col_t = offs[kt_] + jt_ * capv[kt_] + wt_

    xsb = xsf.astype(BF16)
    xtb = xtf.astype(BF16)

    xs_all = np.zeros((N_CORES, D, ct), dtype=np.float16)
    xt_all = np.zeros((N_CORES, D, ct), dtype=np.float16)
    xs_all[cs_, :, col_s] = xsf.astype(np.float16)
    xt_all[ct_, :, col_t] = xtf.astype(np.float16)
    xp_all = np.zeros((N_CORES, 2 * D, ct), dtype=BF16)
    xp_all[cs_, :D, col_s] = xsb
    xp_all[ct_, D:, col_t] = xtb

    vs_all = np.zeros((N_CORES, cap0, GPC, VW), dtype=BF16)
    vt_all = np.zeros((N_CORES, cap0, GPC, VW), dtype=BF16)
    vs_all[cs_, ws_, 4 * ks_ + js_, :D] = xsb
    vs_all[cs_, ws_, 4 * ks_ + js_, D] = 1.0
    vt_all[ct_, wt_, 4 * kt_ + jt_, :D] = xtb
    vt_all[ct_, wt_, 4 * kt_ + jt_, D] = 1.0

    w1a = np.asarray(w1, dtype=np.float32)
    w2a = np.asarray(w2, dtype=np.float32)
    b1a = np.asarray(b1, dtype=np.float32).reshape(D)
    b2a = np.asarray(b2, dtype=np.float32).reshape(D)
    cwb = np.zeros((2 * D, 6 * D), dtype=np.float32)
    cwb[:D, 0:D] = w1a; cwb[D:, D:2 * D] = w1a
    cwb[:D, 2 * D:3 * D] = w2a; cwb[D:, 3 * D:4 * D] = w2a
    cwb[:, 4 * D:6 * D] = np.eye(2 * D, dtype=np.float32)
    cbf = np.stack([np.concatenate([b1a, b1a]),
                    np.concatenate([b2a, b2a])], axis=1)

    in_maps = []
    for c in range(N_CORES):
        in_maps.append({
            "xs": xs_all[c], "xt": xt_all[c], "xp": xp_all[c],
            "vs": np.ascontiguousarray(vs_all[c].reshape(cap0, GPC * VW)),
            "vt": np.ascontiguousarray(vt_all[c].reshape(cap0, GPC * VW)),
            "cwb": cwb.astype(BF16), "cbf": np.ascontiguousarray(cbf),
        })
    meta = (cs_, ct_, col_s, col_t)
    return in_maps, meta


def _numpy_fallback(x_src, batch_src, x_tar, batch_tar, w1, b1, w2, b2):
    bs = np.asarray(batch_src); bt = np.asarray(batch_tar)
    xs = np.asarray(x_src, dtype=np.float64); xt = np.asarray(x_tar, dtype=np.float64)
    mask = bs[:, None] == bt[None, :]

    def attend(q, kv, m):
        s = np.where(m, q @ kv.T, -1.0e9)
        s = s - s.max(axis=1, keepdims=True)
        e = np.exp(s)
        a = e / e.sum(axis=1, keepdims=True)
        out = a @ kv + q
        return np.where(m.any(axis=1, keepdims=True), out, 0.0)

    def mlp(x):
        return np.maximum(x @ w1 + b1, 0.0) @ w2 + b2 + x

    es = mlp(attend(xs, xt, mask))
    et = mlp(attend(xt, xs, mask.T))
    return et.astype(np.float32), es.astype(np.float32)


def kernel(x_src, batch_src, x_tar, batch_tar, w1, b1, w2, b2):
    bs = np.asarray(batch_src).astype(np.int64)
    bt = np.asarray(batch_tar).astype(np.int64)
    if bs.min(initial=0) < 0 or bs.max(initial=0) >= G \
            or bt.min(initial=0) < 0 or bt.max(initial=0) >= G \
            or not (np.all(np.diff(bs) >= 0) and np.all(np.diff(bt) >= 0)):
        return _numpy_fallback(
            x_src, batch_src, x_tar, batch_tar, w1, b1, w2, b2)
    cnt_s = np.bincount(bs, minlength=G)
    cnt_t = np.bincount(bt, minlength=G)
    if max(cnt_s.max(initial=0), cnt_t.max(initial=0)) > 125:
        return _numpy_fallback(
            x_src, batch_src, x_tar, batch_tar, w1, b1, w2, b2)

    plan = _plan(cnt_s, cnt_t)
    caps = plan[0]
    in_maps, meta = _shard_inputs(
        x_src, batch_src, x_tar, batch_tar, w1, b1, w2, b2, plan)
    cs_, ct_, col_s, col_t = meta

    import os
    from concourse import bass_utils
    if caps not in _PROGRAM_CACHE:
        _PROGRAM_CACHE[caps] = _build_program(caps)
    nc = _PROGRAM_CACHE[caps]
    trace = bool(os.environ.get("KERNEL_TRACE"))
    res = bass_utils.run_bass_kernel_spmd(
        nc, in_maps, core_ids=list(range(N_CORES)), trace=trace)
    _PROGRAM_CACHE["last_result"] = res

    ct_cols = 4 * sum(caps)
    cap0 = max(caps)
    offs = [4 * sum(caps[:k]) for k in range(NBAND)]
    outs = np.empty((N_CORES, 2 * D, ct_cols), np.float32)
    for c in range(N_CORES):
        op = np.asarray(res.results[c]["outp"]).reshape(NBAND, 2 * D, 4 * cap0)
        for k in range(NBAND):
            outs[c, :, offs[k]:offs[k] + 4 * caps[k]] = op[k, :, 0:4 * caps[k]]
    embed_src = np.ascontiguousarray(outs[cs_, :D, col_s])
    embed_tar = np.ascontiguousarray(outs[ct_, D:, col_t])
    embed_src[cnt_t[bs] == 0] = 0.0
    embed_tar[cnt_s[bt] == 0] = 0.0
    return embed_tar, embed_src


# revision 18
# speedup vs baseline: 1.1033x; 1.1033x over previous
"""Block-diagonal cross-attention + MLP for trn2, 8-core data-parallel.

v4: size-sorted graph banding + fp16 score matmuls + superpair batching.

- 128 graphs sorted by size desc into 4 bands of 32; band k gets a shared
  cap[k] = roundup(max size, 8). Each core takes 4 graphs from each band
  (one superpair/SP of 4 graphs per band) -> identical shapes across cores
  (SPMD) with ~30% less padding than a global cap.
- Scores in fp16 (4x the mantissa of bf16 at the same PE speed); E, values,
  weights, eT in bf16; PSUM/normalization in fp32; output fp32.
- Per SP: 8 score MMs into a 2-bank psum [cap, 1024] (4 blocks/bank, never
  crossing a bank), ONE exp ACT (strided) -> et bf16, 8 V MMs into the
  psum slot (values + mask-col rowsum), ONE reciprocal, ONE broadcast-AP
  normalize mul -> er bf16, 4 PE transposes -> tp psum, ONE fused
  evict+residual add -> eT.
- MLP over eT in <=512 chunks; bias1+relu alternates ACT/DVE; residual via
  identity matmul; per-chunk output DMA.
- 7 input DMAs + 3 output DMAs total, split across sync/gpsimd queues.
"""

from contextlib import ExitStack

import numpy as np
import ml_dtypes

BF16 = ml_dtypes.bfloat16

N_NODES = 8192
D = 64
G = 128
N_CORES = 8
GPC = G // N_CORES          # graphs per core = 16
VW = D + 1                  # value width incl. mask column
NBAND = 4                   # bands == superpairs per core
BANDG = G // NBAND          # graphs per band = 32

_PROGRAM_CACHE = {}


def _build_program(caps):
    import concourse.bass as bass
    import concourse.tile as tile
    from concourse import bacc, mybir

    fp32 = mybir.dt.float32
    bf16 = mybir.dt.bfloat16
    fp16 = mybir.dt.float16
    caps = list(caps)
    cap0 = max(caps)
    off = [4 * sum(caps[:k]) for k in range(NBAND)]
    ct = 4 * sum(caps)                  # total node columns per core
    nc = bacc.Bacc("TRN2", target_bir_lowering=False, debug=False)

    xs = nc.declare_dram_parameter("xs", [D, ct], fp16, isOutput=False)
    xt = nc.declare_dram_parameter("xt", [D, ct], fp16, isOutput=False)
    xp = nc.declare_dram_parameter("xp", [2 * D, ct], bf16, isOutput=False)
    # vs | vt packed side by side
    vv = nc.declare_dram_parameter("vv", [cap0, 2 * GPC * VW], bf16,
                                   isOutput=False)
    # w1 | w2 | ident packed [128, 384] bf16; b1 | b2 packed [128, 2] fp32
    cwb = nc.declare_dram_parameter("cwb", [2 * D, 6 * D], bf16, isOutput=False)
    cbf = nc.declare_dram_parameter("cbf", [2 * D, 2], fp32, isOutput=False)
    # chunk-major output: band k's chunk lives at rows [128*k, 128*(k+1))
    outp = nc.declare_dram_parameter("outp", [NBAND * 2 * D, 4 * cap0], fp32,
                                     isOutput=True)

    AF = mybir.ActivationFunctionType
    ALU = mybir.AluOpType

    with tile.TileContext(nc) as tc, ExitStack() as ctx:
        singles = ctx.enter_context(tc.tile_pool(name="singles", bufs=1))
        epool = ctx.enter_context(tc.tile_pool(name="epool", bufs=3))
        work = ctx.enter_context(tc.tile_pool(name="work", bufs=3))
        opool = ctx.enter_context(tc.tile_pool(name="opool", bufs=3))

        sb_xs = singles.tile([D, ct], fp16, tag="xs")
        sb_xt = singles.tile([D, ct], fp16, tag="xt")
        sb_xp = singles.tile([2 * D, ct], bf16, tag="xp")
        sb_vv = singles.tile([cap0, 2 * GPC * VW], bf16, tag="vv")
        sb_vs = sb_vv[:, 0:GPC * VW]
        sb_vt = sb_vv[:, GPC * VW:2 * GPC * VW]
        sb_cw = singles.tile([2 * D, 6 * D], bf16, tag="cwb")
        sb_cb = singles.tile([2 * D, 2], fp32, tag="cbf")
        sb_eT = singles.tile([2 * D, ct], bf16, tag="eT")
        sb_h = singles.tile([2 * D, ct], bf16, tag="h")
        sb_w1 = sb_cw[:, 0:2 * D]
        sb_w2 = sb_cw[:, 2 * D:4 * D]
        sb_id = sb_cw[:, 4 * D:6 * D]
        sb_b1 = sb_cb[:, 0:1]
        sb_b2 = sb_cb[:, 1:2]

        # two HWDGE queues (sync + scalar), ordered by first-need time.
        # scalar only gets 4 early issues so exp_0 isn't delayed.
        o1 = off[1]

        def vv_band(t, k):
            # both the vs and vt slices of band k in one 3D access pattern
            return t.rearrange("p (s c) -> p s c", s=2)[
                :, :, 4 * k * VW:4 * (k + 1) * VW]

        nc.sync.dma_start(out=sb_xs[:, 0:o1], in_=xs[:, 0:o1])
        nc.scalar.dma_start(out=sb_xt[:, 0:o1], in_=xt[:, 0:o1])
        nc.sync.dma_start(out=sb_xs[:, o1:ct], in_=xs[:, o1:ct])
        nc.scalar.dma_start(out=sb_xt[:, o1:ct], in_=xt[:, o1:ct])
        nc.sync.dma_start(out=vv_band(sb_vv, 0), in_=vv_band(vv, 0))
        nc.scalar.dma_start(out=vv_band(sb_vv, 1), in_=vv_band(vv, 1))
        nc.sync.dma_start(out=sb_cw, in_=cwb[:, :])
        nc.scalar.dma_start(out=vv_band(sb_vv, 3), in_=vv_band(vv, 3))
        nc.sync.dma_start(out=sb_xp[:, 0:o1], in_=xp[:, 0:o1])
        nc.sync.dma_start(out=vv_band(sb_vv, 2), in_=vv_band(vv, 2))
        nc.sync.dma_start(out=sb_xp[:, o1:ct], in_=xp[:, o1:ct])
        nc.sync.dma_start(out=sb_cb, in_=cbf[:, :])

        def scol(cap, j, d):
            return 512 * (j // 2) + ((j % 2) * 2 + d) * cap

        def ocol(j, d):
            return 512 * (j // 2) + ((j % 2) * 2 + d) * VW

        scs, ets, ers = {}, {}, {}
        with tc.tile_pool(name="ps_a", bufs=2, space="PSUM") as ps_a, \
             tc.tile_pool(name="ps_t", bufs=2, space="PSUM") as ps_t, \
             tc.tile_pool(name="ps_m", bufs=2, space="PSUM") as ps_m:
            # skewed pipeline: iter k emits scores_k/exp_k, V_{k-1},
            # tp/add/mlp-L1_{k-2}, mlp-L2_{k-3} so the PE stream never
            # waits on the scalar/vector stages
            for k in range(NBAND + 3):
                if k < NBAND:
                    cap = caps[k]
                    ob = off[k]
                    sc = ps_a.tile([cap, 1024], fp32, tag="sco")
                    scs[k] = sc
                    for j in range(4):
                        xsj = sb_xs[:, ob + j * cap: ob + (j + 1) * cap]
                        xtj = sb_xt[:, ob + j * cap: ob + (j + 1) * cap]
                        nc.tensor.matmul(sc[:, scol(cap, j, 0):scol(cap, j, 0) + cap],
                                         xsj, xtj, start=True, stop=True)
                        nc.tensor.matmul(sc[:, scol(cap, j, 1):scol(cap, j, 1) + cap],
                                         xtj, xsj, start=True, stop=True)
                    et = epool.tile([cap, 8 * cap], bf16, tag="E")
                    ets[k] = et
                    sc_str = sc.rearrange("p (a c) -> p a c", a=2)[:, :, 0:4 * cap]
                    nc.scalar.activation(out=et, in_=sc_str, func=AF.Exp)

                if 1 <= k <= NBAND:
                    kk = k - 1
                    cap = caps[kk]
                    sc, et = scs[kk], ets[kk]
                    # V matmuls reuse the score psum slot (WAR on exp)
                    for j in range(4):
                        vcol = (4 * kk + j) * VW
                        nc.tensor.matmul(
                            sc[:, ocol(j, 0):ocol(j, 0) + VW],
                            et[:, (2 * j + 1) * cap:(2 * j + 2) * cap],
                            sb_vt[0:cap, vcol:vcol + VW], start=True, stop=True)
                        nc.tensor.matmul(
                            sc[:, ocol(j, 1):ocol(j, 1) + VW],
                            et[:, (2 * j) * cap:(2 * j + 1) * cap],
                            sb_vs[0:cap, vcol:vcol + VW], start=True, stop=True)
                    o4 = sc.rearrange("p (a c) -> p a c", a=2)[:, :, 0:4 * VW] \
                           .rearrange("p a (b w) -> p a b w", w=VW)
                    rc = work.tile([cap, 8], fp32, tag="rc")
                    nc.vector.reciprocal(out=rc, in_=o4[:, :, 0:4, D:D + 1])
                    er = work.tile([cap, 8 * D], bf16, tag="er")
                    ers[kk] = er
                    rcb = rc.rearrange("p (a b) -> p a b", a=2).to_broadcast(
                        [cap, 2, 4, D])
                    nc.vector.tensor_mul(er.rearrange("p (a b w) -> p a b w",
                                                      a=2, w=D),
                                         o4[:, :, 0:4, 0:D], rcb)

                if 2 <= k <= NBAND + 1:
                    kk = k - 2
                    cap = caps[kk]
                    ob = off[kk]
                    er = ers.pop(kk)
                    tp = ps_t.tile([2 * D, 4 * cap], bf16, tag="tp")
                    for j in range(4):
                        nc.tensor.transpose(tp[:, j * cap:(j + 1) * cap],
                                            er[:, j * 2 * D:(j + 1) * 2 * D],
                                            sb_id[0:cap, 0:cap])
                    nc.vector.tensor_add(sb_eT[:, ob:ob + 4 * cap], tp,
                                         sb_xp[:, ob:ob + 4 * cap])
                    # MLP layer 1 for this band's chunk
                    w = 4 * cap
                    hp = ps_m.tile([2 * D, 4 * cap0], fp32, tag="m")
                    nc.tensor.matmul(hp[:, 0:w], sb_w1, sb_eT[:, ob:ob + w],
                                     start=True, stop=True)
                    if kk % 2 == 0:
                        nc.scalar.activation(out=sb_h[:, ob:ob + w],
                                             in_=hp[:, 0:w],
                                             func=AF.Relu, bias=sb_b1, scale=1.0)
                    else:
                        nc.vector.tensor_scalar(
                            out=sb_h[:, ob:ob + w], in0=hp[:, 0:w],
                            scalar1=sb_b1,
                            scalar2=0.0, op0=ALU.add, op1=ALU.max)

                if k >= 3:
                    kk = k - 3
                    cap = caps[kk]
                    ob = off[kk]
                    w = 4 * cap
                    op2 = ps_m.tile([2 * D, 4 * cap0], fp32, tag="m")
                    nc.tensor.matmul(op2[:, 0:w], sb_w2, sb_h[:, ob:ob + w],
                                     start=True, stop=False)
                    nc.tensor.matmul(op2[:, 0:w], sb_id, sb_eT[:, ob:ob + w],
                                     start=False, stop=True)
                    ot = opool.tile([2 * D, 4 * cap0], fp32, tag="out")
                    if kk % 2 == 0:
                        nc.scalar.activation(out=ot[:, 0:w], in_=op2[:, 0:w],
                                             func=AF.Identity, bias=sb_b2,
                                             scale=1.0)
                    else:
                        nc.vector.tensor_scalar_add(ot[:, 0:w], op2[:, 0:w],
                                                    sb_b2)
                    nc.sync.dma_start(out=outp[kk * 2 * D:(kk + 1) * 2 * D, 0:w],
                                      in_=ot[:, 0:w])

    nc.compile()
    return nc


def _plan(cnt_s, cnt_t):
    size = np.maximum(cnt_s, cnt_t)
    order = np.argsort(-size, kind="stable")
    bands = order.reshape(NBAND, BANDG)
    caps = tuple(int(-(-int(size[b].max()) // 8) * 8) for b in bands)
    core_of = np.empty(G, np.int64)
    band_of = np.empty(G, np.int64)
    slot_of = np.empty(G, np.int64)
    for k in range(NBAND):
        for c in range(N_CORES):
            for j in range(4):
                g = bands[k, c * 4 + j]
                core_of[g] = c
                band_of[g] = k
                slot_of[g] = j
    return caps, core_of, band_of, slot_of


def _shard_inputs(x_src, batch_src, x_tar, batch_tar, w1, b1, w2, b2, plan):
    caps, core_of, band_of, slot_of = plan
    bs = np.asarray(batch_src).astype(np.int64)
    bt = np.asarray(batch_tar).astype(np.int64)
    xsf = np.asarray(x_src, dtype=np.float32)
    xtf = np.asarray(x_tar, dtype=np.float32)
    cap0 = max(caps)
    offs = np.array([4 * sum(caps[:k]) for k in range(NBAND)], np.int64)
    capv = np.array(caps, np.int64)
    ct = int(4 * sum(caps))

    bnd_s = np.searchsorted(bs, np.arange(G + 1))
    bnd_t = np.searchsorted(bt, np.arange(G + 1))
    ws_ = np.arange(N_NODES) - bnd_s[bs]
    wt_ = np.arange(N_NODES) - bnd_t[bt]
    cs_, ct_ = core_of[bs], core_of[bt]
    ks_, kt_ = band_of[bs], band_of[bt]
    js_, jt_ = slot_of[bs], slot_of[bt]
    col_s = offs[ks_] + js_ * capv[ks_] + ws_
    col_t = offs[kt_] + jt_ * capv[kt_] + wt_

    xsb = xsf.astype(BF16)
    xtb = xtf.astype(BF16)

    xs_all = np.zeros((N_CORES, D, ct), dtype=np.float16)
    xt_all = np.zeros((N_CORES, D, ct), dtype=np.float16)
    xs_all[cs_, :, col_s] = xsf.astype(np.float16)
    xt_all[ct_, :, col_t] = xtf.astype(np.float16)
    xp_all = np.zeros((N_CORES, 2 * D, ct), dtype=BF16)
    xp_all[cs_, :D, col_s] = xsb
    xp_all[ct_, D:, col_t] = xtb

    vs_all = np.zeros((N_CORES, cap0, GPC, VW), dtype=BF16)
    vt_all = np.zeros((N_CORES, cap0, GPC, VW), dtype=BF16)
    vs_all[cs_, ws_, 4 * ks_ + js_, :D] = xsb
    vs_all[cs_, ws_, 4 * ks_ + js_, D] = 1.0
    vt_all[ct_, wt_, 4 * kt_ + jt_, :D] = xtb
    vt_all[ct_, wt_, 4 * kt_ + jt_, D] = 1.0

    w1a = np.asarray(w1, dtype=np.float32)
    w2a = np.asarray(w2, dtype=np.float32)
    b1a = np.asarray(b1, dtype=np.float32).reshape(D)
    b2a = np.asarray(b2, dtype=np.float32).reshape(D)
    cwb = np.zeros((2 * D, 6 * D), dtype=np.float32)
    cwb[:D, 0:D] = w1a; cwb[D:, D:2 * D] = w1a
    cwb[:D, 2 * D:3 * D] = w2a; cwb[D:, 3 * D:4 * D] = w2a
    cwb[:, 4 * D:6 * D] = np.eye(2 * D, dtype=np.float32)
    cbf = np.stack([np.concatenate([b1a, b1a]),
                    np.concatenate([b2a, b2a])], axis=1)

    in_maps = []
    for c in range(N_CORES):
        in_maps.append({
            "xs": xs_all[c], "xt": xt_all[c], "xp": xp_all[c],
            "vv": np.ascontiguousarray(np.concatenate(
                [vs_all[c].reshape(cap0, GPC * VW),
                 vt_all[c].reshape(cap0, GPC * VW)], axis=1)),
            "cwb": cwb.astype(BF16), "cbf": np.ascontiguousarray(cbf),
        })
    meta = (cs_, ct_, col_s, col_t)
    return in_maps, meta


def _numpy_fallback(x_src, batch_src, x_tar, batch_tar, w1, b1, w2, b2):
    bs = np.asarray(batch_src); bt = np.asarray(batch_tar)
    xs = np.asarray(x_src, dtype=np.float64); xt = np.asarray(x_tar, dtype=np.float64)
    mask = bs[:, None] == bt[None, :]

    def attend(q, kv, m):
        s = np.where(m, q @ kv.T, -1.0e9)
        s = s - s.max(axis=1, keepdims=True)
        e = np.exp(s)
        a = e / e.sum(axis=1, keepdims=True)
        out = a @ kv + q
        return np.where(m.any(axis=1, keepdims=True), out, 0.0)

    def mlp(x):
        return np.maximum(x @ w1 + b1, 0.0) @ w2 + b2 + x

    es = mlp(attend(xs, xt, mask))
    et = mlp(attend(xt, xs, mask.T))
    return et.astype(np.float32), es.astype(np.float32)


def kernel(x_src, batch_src, x_tar, batch_tar, w1, b1, w2, b2):
    bs = np.asarray(batch_src).astype(np.int64)
    bt = np.asarray(batch_tar).astype(np.int64)
    if bs.min(initial=0) < 0 or bs.max(initial=0) >= G \
            or bt.min(initial=0) < 0 or bt.max(initial=0) >= G \
            or not (np.all(np.diff(bs) >= 0) and np.all(np.diff(bt) >= 0)):
        return _numpy_fallback(
            x_src, batch_src, x_tar, batch_tar, w1, b1, w2, b2)
    cnt_s = np.bincount(bs, minlength=G)
    cnt_t = np.bincount(bt, minlength=G)
    if max(cnt_s.max(initial=0), cnt_t.max(initial=0)) > 125:
        return _numpy_fallback(
            x_src, batch_src, x_tar, batch_tar, w1, b1, w2, b2)

    plan = _plan(cnt_s, cnt_t)
    caps = plan[0]
    in_maps, meta = _shard_inputs(
        x_src, batch_src, x_tar, batch_tar, w1, b1, w2, b2, plan)
    cs_, ct_, col_s, col_t = meta

    import os
    from concourse import bass_utils
    if caps not in _PROGRAM_CACHE:
        _PROGRAM_CACHE[caps] = _build_program(caps)
    nc = _PROGRAM_CACHE[caps]
    trace = bool(os.environ.get("KERNEL_TRACE"))
    res = bass_utils.run_bass_kernel_spmd(
        nc, in_maps, core_ids=list(range(N_CORES)), trace=trace)
    _PROGRAM_CACHE["last_result"] = res

    ct_cols = 4 * sum(caps)
    cap0 = max(caps)
    offs = [4 * sum(caps[:k]) for k in range(NBAND)]
    outs = np.empty((N_CORES, 2 * D, ct_cols), np.float32)
    for c in range(N_CORES):
        op = np.asarray(res.results[c]["outp"]).reshape(NBAND, 2 * D, 4 * cap0)
        for k in range(NBAND):
            outs[c, :, offs[k]:offs[k] + 4 * caps[k]] = op[k, :, 0:4 * caps[k]]
    embed_src = np.ascontiguousarray(outs[cs_, :D, col_s])
    embed_tar = np.ascontiguousarray(outs[ct_, D:, col_t])
    embed_src[cnt_t[bs] == 0] = 0.0
    embed_tar[cnt_s[bt] == 0] = 0.0
    return embed_tar, embed_src
